# revision 39
# baseline (speedup 1.0000x reference)
"""GAT (2-layer) + MLP head on 8 TRN2 NeuronCores.

Strategy
--------
The random edge list (320k edges over 1600 nodes) is converted on the host
into a dense edge-count matrix C [dst, src] (a lossless re-layout of
edge_index: C[d,s] = number of (s->d) edges).  The GAT edge softmax then
becomes dense row-wise ops + matmuls:

    e(s,d)   = leaky_relu(asrc[s] + adst[d])          (rank-1 structure)
    P[d,s]   = C[d,s] * exp(e(s,d))                   (no max-sub needed; |e|<~3)
    denom[d] = sum_s P[d,s]
    out[d]   = (P @ h)[d] / denom[d]

Sharding: each core owns 25 destination nodes of each of the 8 graphs
(dst-interleaved).  That makes the final FC layer (fc1_w [200, 200*512])
column-shardable with a tiny [8,200] AllReduce, while layer1->layer2
needs one AllGather of the 2-layer hidden state ([200, 514] bf16/core).

All matmuls run in bf16 with fp32 PSUM accumulation.
"""

import sys
import numpy as np

sys.path.insert(0, "/opt/trn_rl_repo")

import ml_dtypes  # noqa: E402

import concourse.bass as bass  # noqa: E402
from concourse import bacc  # noqa: E402
from concourse import mybir  # noqa: E402
from concourse.tile import TileContext  # noqa: E402
from concourse.bass_utils import run_bass_kernel_spmd  # noqa: E402

# ---------------------------------------------------------------- constants
N = 1600
ROI = 200
HID = 64
HIN = 8
D1 = HID * HIN  # 512
B = 8
NCORES = 8
NEG = 0.2
NODES_PER_CORE = N // NCORES       # 200
PER_GRAPH = NODES_PER_CORE // B    # 25

F32 = mybir.dt.float32
BF16 = mybir.dt.bfloat16

import os  # noqa: E402

DEBUG_STAGE = os.environ.get("KERNEL_DEBUG_STAGE") or None
DEBUG_HEADS = int(os.environ.get("KERNEL_DEBUG_HEADS") or HIN)

# node k-tiles over the 1600-node dim
KT = [(t * 128, min(128, N - t * 128)) for t in range((N + 127) // 128)]  # 13
NKT = len(KT)

_BF = ml_dtypes.bfloat16


def _bf(x):
    return np.ascontiguousarray(x.astype(_BF))


def _f32(x):
    return np.ascontiguousarray(x.astype(np.float32))


def _ap_cols(ap, start, stride, count):
    """Sub-AP selecting `count` columns with `stride` from a 2D [P, F] AP."""
    return bass.AP(
        tensor=ap.tensor,
        offset=ap.offset + start * ap.ap[-1][0],
        ap=[ap.ap[0], [ap.ap[-1][0] * stride, count]],
    )


def _bcast_free(ap, n):
    """[P, 1] AP -> [P, n] with stride-0 free dim."""
    return bass.AP(tensor=ap.tensor, offset=ap.offset, ap=[ap.ap[0], [0, n]])


def _dram_bcast(handle, n_part, offset, stride, count):
    """DRAM read AP replicating a strided 1-D slice across n_part partitions."""
    return bass.AP(
        tensor=handle, offset=offset, ap=[[0, n_part], [stride, count]]
    )


# ---------------------------------------------------------------- program
def build_program():
    nc = bacc.Bacc("TRN2", num_devices=NCORES)

    # ---- I/O ----
    d_xT = nc.dram_tensor("xT", [100, 2, N], BF16, kind="ExternalInput")
    d_xTd = nc.dram_tensor("xTd", [100, 2, NODES_PER_CORE], BF16, kind="ExternalInput")
    d_W1 = nc.dram_tensor("W1", [100, 2, D1], BF16, kind="ExternalInput")
    d_Wa = nc.dram_tensor("Wa", [100, 2, 16], BF16, kind="ExternalInput")
    d_Ct1 = nc.dram_tensor("Ct1", [128, NKT, NODES_PER_CORE], BF16, kind="ExternalInput")
    d_Ct2 = nc.dram_tensor("Ct2", [128, NKT, NODES_PER_CORE], BF16, kind="ExternalInput")
    d_b1 = nc.dram_tensor("b1", [D1], F32, kind="ExternalInput")
    d_b2c = nc.dram_tensor("b2c", [128, 4], F32, kind="ExternalInput")
    d_Wb = nc.dram_tensor("Wb", [2, D1], F32, kind="ExternalInput")
    d_W2 = nc.dram_tensor("W2", [128, 4, D1], BF16, kind="ExternalInput")
    d_WfcT = nc.dram_tensor("WfcT", [128, 100, ROI], BF16, kind="ExternalInput")
    d_bnsc = nc.dram_tensor("bnsc", [100, 2], F32, kind="ExternalInput")
    d_bnsh = nc.dram_tensor("bnsh", [100, 2], F32, kind="ExternalInput")
    d_fc2wT = nc.dram_tensor("fc2wT", [100, 2, 2], BF16, kind="ExternalInput")
    d_fc2b = nc.dram_tensor("fc2b", [2], F32, kind="ExternalInput")
    d_out = nc.dram_tensor("logits", [B, 2], F32, kind="ExternalOutput")
    d_dbg = nc.dram_tensor("dbg", [128, 800], F32, kind="ExternalOutput")

    # ---- collective buffers ----
    AGW = D1 + 2  # 514: [features | asrc2 | adst2]
    d_ag_in = nc.dram_tensor("ag_in", [NODES_PER_CORE, AGW], BF16, kind="Internal")
    d_ag_out = nc.dram_tensor(
        "ag_out", [N, AGW], BF16, kind="Internal", addr_space="Shared"
    )
    d_ar_in = nc.dram_tensor("ar_in", [B, ROI], F32, kind="Internal")
    d_ar_out = nc.dram_tensor(
        "ar_out", [B, ROI], F32, kind="Internal", addr_space="Shared"
    )
    d_asdT = nc.dram_tensor("asdT_scratch", [16, NODES_PER_CORE], BF16, kind="Internal")

    groups = [list(range(NCORES))]

    with TileContext(nc) as tc:
        _build_body(nc, tc, locals())

    nc.finalize()

    in_names = [
        "xT", "xTd", "W1", "Wa", "Ct1", "Ct2", "b1", "b2c", "Wb", "W2",
        "WfcT", "bnsc", "bnsh", "fc2wT", "fc2b",
    ]
    return nc, in_names


def _build_body(nc, tc, d):
    from contextlib import ExitStack

    d_xT = d["d_xT"]; d_xTd = d["d_xTd"]; d_W1 = d["d_W1"]; d_Wa = d["d_Wa"]
    d_Ct1 = d["d_Ct1"]; d_Ct2 = d["d_Ct2"]; d_b1 = d["d_b1"]; d_b2c_d = d["d_b2c"]
    d_Wb = d["d_Wb"]; d_W2 = d["d_W2"]; d_WfcT = d["d_WfcT"]
    d_bnsc = d["d_bnsc"]; d_bnsh = d["d_bnsh"]; d_fc2wT = d["d_fc2wT"]
    d_fc2b = d["d_fc2b"]; d_out = d["d_out"]
    d_ag_in = d["d_ag_in"]; d_ag_out = d["d_ag_out"]
    d_ar_in = d["d_ar_in"]; d_ar_out = d["d_ar_out"]
    d_asdT = d["d_asdT"]
    d_dbg = d["d_dbg"]
    groups = d["groups"]

    ACT = mybir.ActivationFunctionType
    ALU = mybir.AluOpType

    def _dbg_out(work, src_ap):
        dbg = work.tile([B, 2], F32, tag="dbg", name="dbg")
        nc.vector.tensor_copy(dbg, src_ap)
        nc.sync.dma_start(out=d_out[:], in_=dbg)

    with ExitStack() as ctx:
        singles = ctx.enter_context(tc.tile_pool(name="singles", bufs=1))
        work = ctx.enter_context(tc.tile_pool(name="work", bufs=3))

        # ------------------------------------------------ static loads
        ones_row = singles.tile([1, 128], BF16)
        nc.vector.memset(ones_row, 1.0)
        ones_col = singles.tile([128, 1], BF16)
        nc.vector.memset(ones_col, 1.0)

        xT = singles.tile([100, 2, N], BF16)           # x^T k-tiles (K=200=2x100)
        nc.sync.dma_start(out=xT[:], in_=d_xT[:])
        xTd = singles.tile([100, 2, NODES_PER_CORE], BF16)
        nc.sync.dma_start(out=xTd[:], in_=d_xTd[:])
        W1 = singles.tile([100, 2, D1], BF16)
        nc.sync.dma_start(out=W1[:], in_=d_W1[:])
        Wa = singles.tile([100, 2, 16], BF16)
        nc.sync.dma_start(out=Wa[:], in_=d_Wa[:])

        Ct1 = singles.tile([128, NKT, NODES_PER_CORE], BF16)
        Ct2 = singles.tile([128, NKT, NODES_PER_CORE], BF16)
        nc.sync.dma_start(out=Ct1[:], in_=d_Ct1[:])

        b1b = singles.tile([128, D1], F32)  # b1 broadcast across partitions
        nc.sync.dma_start(out=b1b, in_=_dram_bcast(d_b1, 128, 0, 1, D1))
        Wbb = singles.tile([128, 2, D1], F32)  # wsrc2 / wdst2 broadcast
        nc.sync.dma_start(
            out=Wbb,
            in_=bass.AP(tensor=d_Wb, offset=0, ap=[[0, 128], [D1, 2], [1, D1]]),
        )
        b2c = singles.tile([128, 4], F32)  # b2 in 4 chunks of 128 (per-partition)
        W2sb = singles.tile([128, 4, D1], BF16)
        WfcT = singles.tile([128, 100, ROI], BF16)
        bnsc = singles.tile([100, 2], F32)
        bnsh = singles.tile([100, 2], F32)
        fc2wT = singles.tile([100, 2, 2], BF16)
        fc2b = singles.tile([B, 2], F32)

        def _load_late_inputs():
            nc.sync.dma_start(out=b2c[:], in_=d_b2c_d[:])
            nc.sync.dma_start(out=W2sb[:], in_=d_W2[:])
            nc.sync.dma_start(out=WfcT[:], in_=d_WfcT[:])
            nc.sync.dma_start(out=bnsc[:], in_=d_bnsc[:])
            nc.sync.dma_start(out=bnsh[:], in_=d_bnsh[:])
            nc.sync.dma_start(out=fc2wT[:], in_=d_fc2wT[:])
            nc.sync.dma_start(out=fc2b, in_=_dram_bcast(d_fc2b, B, 0, 1, 2))
            nc.sync.dma_start(out=Ct2[:], in_=d_Ct2[:])

        # ------------------------------------------------ phase A: asdT1 + h1
        h1s = singles.tile([128, NKT, HIN, HID + 1], BF16)
        nc.vector.memset(h1s[:, :, :, HID : HID + 1], 1.0)
        asd1 = singles.tile([128, NKT, 16], F32)
        adstb = singles.tile([128, HIN, NODES_PER_CORE], BF16)
        with tc.tile_pool(name="pA", bufs=2, space="PSUM") as pA:
            # asdT1 = Wa^T @ x[D_j]^T  (row h = asrc1_h, row 8+h = adst1_h)
            ps_asdT = pA.tile([16, NODES_PER_CORE], F32)
            for kc in range(2):
                nc.tensor.matmul(
                    ps_asdT, Wa[:, kc, :], xTd[:, kc, :],
                    start=(kc == 0), stop=(kc == 1),
                )
            asdT1 = work.tile([16, NODES_PER_CORE], BF16, tag="asdT1")
            nc.vector.tensor_copy(asdT1, ps_asdT)
            nc.sync.dma_start(out=d_asdT[:], in_=asdT1)
            # broadcast all heads' adst rows across 128 partitions via DRAM re-read
            nc.sync.dma_start(
                out=adstb[:],
                in_=bass.AP(
                    tensor=d_asdT,
                    offset=8 * NODES_PER_CORE,
                    ap=[[0, 128], [NODES_PER_CORE, HIN], [1, NODES_PER_CORE]],
                ),
            )

            # h1 = x @ W1 (+ asd1);  h1s[:, k, h, 64] stays 1.0 (denominator col)
            for k, (k0, pk) in enumerate(KT):
                ps_h = pA.tile([128, D1], F32, tag="ps_h")
                ps_a = pA.tile([128, 16], F32, tag="ps_a")
                for kc in range(2):
                    nc.tensor.matmul(
                        ps_h[0:pk, :],
                        xT[:, kc, k0 : k0 + pk],
                        W1[:, kc, :],
                        start=(kc == 0),
                        stop=(kc == 1),
                    )
                    nc.tensor.matmul(
                        ps_a[0:pk, :],
                        xT[:, kc, k0 : k0 + pk],
                        Wa[:, kc, :],
                        start=(kc == 0),
                        stop=(kc == 1),
                    )
                pa = ps_h[0:pk, :]
                nc.vector.tensor_copy(
                    h1s[0:pk, k, :, 0:HID],
                    bass.AP(
                        tensor=pa.tensor,
                        offset=pa.offset,
                        ap=[pa.ap[0], [HID, HIN], [1, HID]],
                    ),
                )
                nc.scalar.copy(asd1[0:pk, k, :], ps_a[0:pk, :])

        if DEBUG_STAGE == "h1":
            dbh = work.tile([128, 536], F32, tag="dbh", name="dbh")
            nc.vector.tensor_copy(dbh[:, 0:520], h1s[:, 0, :, :])
            nc.vector.tensor_copy(dbh[:, 520:536], asd1[:, 0, :])
            nc.sync.dma_start(out=d_dbg[:, 0:536], in_=dbh)
            _dbg_out(work, asd1[0:B, 0, 0:2])
            return

        # ------------------------------------------------ layer-1 attention
        run_mm = DEBUG_STAGE != "l1a"
        run_post = DEBUG_STAGE not in ("l1a", "l1b")
        g1f = singles.tile([100, 2, D1 + 2], F32)  # per-m: [feat 512 | asrc2 adst2]
        with tc.tile_pool(name="pL1", bufs=2, space="PSUM") as pL1:
            for h in range(DEBUG_HEADS):
                if run_mm:
                    psA = pL1.tile([100, HID + 1], F32, tag="psA")
                    psB = pL1.tile([100, HID + 1], F32, tag="psB")
                ebuf = work.tile([128, NKT, NODES_PER_CORE], F32, tag="ebuf", bufs=2)
                for k, (k0, pk) in enumerate(KT):
                    nc.scalar.activation(
                        ebuf[0:pk, k, :],
                        adstb[0:pk, h, :],
                        ACT.Prelu,
                        bias=asd1[0:pk, k, h : h + 1],
                        scale=1.0,
                        alpha=NEG,
                    )
                nc.scalar.activation(ebuf[:], ebuf[:], ACT.Exp)
                pt = work.tile([128, NKT, NODES_PER_CORE], BF16, tag="pt", bufs=2)
                nc.vector.tensor_mul(pt[:], ebuf[:], Ct1[:])
                if not run_mm:
                    if h == 0:
                        _dbg_out(work, pt[0:B, 0, 0:2])
                    continue
                for k, (k0, pk) in enumerate(KT):
                    nc.tensor.matmul(
                        psA,
                        pt[0:pk, k, 0:100],
                        h1s[0:pk, k, h, :],
                        start=(k == 0),
                        stop=(k == NKT - 1),
                    )
                    nc.tensor.matmul(
                        psB,
                        pt[0:pk, k, 100:200],
                        h1s[0:pk, k, h, :],
                        start=(k == 0),
                        stop=(k == NKT - 1),
                    )
                if not run_post:
                    if run_mm and h == 0:
                        _dbg_out(work, psA[0:B, 0:2])
                    continue
                for m, ps in ((0, psA), (1, psB)):
                    rec = work.tile([100, 1], F32, tag="rec")
                    nc.vector.tensor_scalar_add(rec, ps[:, HID : HID + 1], 1e-16)
                    nc.vector.reciprocal(rec, rec)
                    vh = g1f[:, m, h * HID : (h + 1) * HID]
                    nc.vector.tensor_scalar_mul(vh, ps[:, 0:HID], rec[:, 0:1])
                if h in (3, HIN - 1):
                    # ELU for the completed half (cols), both m-halves
                    c0 = 0 if h == 3 else 256
                    for m in range(2):
                        vv = g1f[:, m, c0 : c0 + 256]
                        nc.vector.tensor_add(vv, vv, b1b[0:100, c0 : c0 + 256])
                        eneg = work.tile([100, 256], F32, tag="eneg")
                        nc.vector.tensor_scalar_min(eneg, vv, 0.0)
                        nc.scalar.activation(eneg, eneg, ACT.Exp)
                        nc.vector.tensor_scalar(vv, vv, 0.0, -1.0, ALU.max, ALU.add)
                        nc.vector.tensor_add(vv, vv, eneg)
                        if h == 3:
                            g1o1 = singles.tile([100, 2, 256], BF16)
                            nc.vector.tensor_copy(g1o1[:, m, :], vv)
                            nc.sync.dma_start(
                                out=d_ag_in[m * 100 : (m + 1) * 100, 0:256],
                                in_=g1o1[:, m, :],
                            )
                if h == 3:
                    _load_late_inputs()
        if DEBUG_STAGE in ("l1a", "l1b"):
            return
        if DEBUG_STAGE == "l1p":
            _dbg_out(work, g1f[0:B, 0, 0:2])
            return

        # ------------------------------------------------ g1 post: asd2 dots
        g1o2 = singles.tile([100, 2, D1 + 2 - 256], BF16)
        for m in range(2):
            v = g1f[:, m, 0:D1]
            sc2 = work.tile([100, D1], F32, tag="sc2")
            for vi in range(2):
                nc.vector.tensor_mul(sc2, v, Wbb[0:100, vi, :])
                nc.vector.tensor_reduce(
                    g1f[:, m, D1 + vi : D1 + vi + 1],
                    sc2,
                    mybir.AxisListType.X,
                    ALU.add,
                )
            nc.vector.tensor_copy(g1o2[:, m, :], g1f[:, m, 256:])
            nc.sync.dma_start(
                out=d_ag_in[m * 100 : (m + 1) * 100, 256:], in_=g1o2[:, m, :]
            )

        if DEBUG_STAGE == "l1":
            nc.sync.dma_start(out=d_dbg[0:100, 0:514], in_=g1f[:, 0, :])
            dbl = work.tile([128, 200], F32, tag="dbl", name="dbl")
            nc.vector.tensor_copy(dbl, adstb[:, 0, :])
            nc.sync.dma_start(out=d_dbg[:, 514:714], in_=dbl)
            _dbg_out(work, g1f[0:B, 0, 0:2])
            return

        # ------------------------------------------------ AllGather
        nc.gpsimd.collective_compute(
            "AllGather",
            ALU.bypass,
            replica_groups=groups,
            ins=[d_ag_in[:]],
            outs=[d_ag_out[:]],
        )

        # g1a: gathered -> [128, k, 514]
        g1a = singles.tile([128, NKT, D1 + 2], BF16)
        AGW2 = D1 + 2
        nc.sync.dma_start(
            out=g1a[:, 0:12, :],
            in_=bass.AP(
                tensor=d_ag_out,
                offset=0,
                ap=[[AGW2, 128], [AGW2 * 128, 12], [1, AGW2]],
            ),
        )
        nc.sync.dma_start(
            out=g1a[0:64, 12, :],
            in_=bass.AP(
                tensor=d_ag_out,
                offset=12 * 128 * AGW2,
                ap=[[AGW2, 64], [1, AGW2]],
            ),
        )
        # own shard's adst2 broadcast across partitions (from local ag_in)
        adst2b = singles.tile([128, NODES_PER_CORE], BF16)
        nc.sync.dma_start(
            out=adst2b,
            in_=bass.AP(
                tensor=d_ag_in,
                offset=AGW2 - 1,
                ap=[[0, 128], [AGW2, NODES_PER_CORE]],
            ),
        )
        asrc2f = singles.tile([128, NKT], F32)
        nc.scalar.copy(asrc2f[:, :], g1a[:, :, D1])

        if DEBUG_STAGE == "ag":
            dba = work.tile([128, 514], F32, tag="dba", name="dba")
            nc.vector.tensor_copy(dba, g1a[:, 0, :])
            nc.sync.dma_start(out=d_dbg[:, 0:514], in_=dba)
            _dbg_out(work, g1a[0:B, 0, 0:2])
            return

        # ------------------------------------------------ layer-2 attention + W2
        out2T = singles.tile([128, 4, NODES_PER_CORE], BF16)
        with tc.tile_pool(name="pL2", bufs=1, space="PSUM") as pL2:
            psT = [
                pL2.tile([128, NODES_PER_CORE], F32, tag=f"psT{c}", name=f"psT{c}")
                for c in range(4)
            ]
            psD = pL2.tile([1, NODES_PER_CORE], F32, tag="psD")
            ebuf2 = work.tile([128, NKT, NODES_PER_CORE], F32, tag="ebuf2", bufs=1)
            for k, (k0, pk) in enumerate(KT):
                nc.scalar.activation(
                    ebuf2[0:pk, k, :],
                    adst2b[0:pk, :],
                    ACT.Prelu,
                    bias=asrc2f[0:pk, k : k + 1],
                    scale=1.0,
                    alpha=NEG,
                )
            nc.scalar.activation(ebuf2[:], ebuf2[:], ACT.Exp)
            pt2 = work.tile([128, NKT, NODES_PER_CORE], BF16, tag="pt2", bufs=1)
            nc.vector.tensor_mul(pt2[:], ebuf2[:], Ct2[:])
            for k, (k0, pk) in enumerate(KT):
                for c in range(4):
                    nc.tensor.matmul(
                        psT[c],
                        g1a[0:pk, k, c * 128 : (c + 1) * 128],
                        pt2[0:pk, k, :],
                        start=(k == 0),
                        stop=(k == NKT - 1),
                    )
                nc.tensor.matmul(
                    psD,
                    ones_col[0:pk, :],
                    pt2[0:pk, k, :],
                    start=(k == 0),
                    stop=(k == NKT - 1),
                )

            # denominator reciprocal, broadcast across partitions via PE
            d2 = work.tile([1, NODES_PER_CORE], F32, tag="d2")
            nc.vector.tensor_scalar_add(d2, psD, 1e-16)
            nc.vector.reciprocal(d2, d2)
            d2b = work.tile([1, NODES_PER_CORE], BF16, tag="d2b")
            nc.vector.tensor_copy(d2b, d2)
            ps_rb = pL2.tile([128, NODES_PER_CORE], F32, tag="ps_rb")
            nc.tensor.matmul(ps_rb, ones_row, d2b)
            rb = work.tile([128, NODES_PER_CORE], F32, tag="rb")
            nc.scalar.copy(rb, ps_rb)

            # tT (bf16 copy of layer-2 message sums), then s^T = W2^T @ t^T
            tT = work.tile([128, 4, NODES_PER_CORE], BF16, tag="tT")
            for c in range(4):
                nc.vector.tensor_copy(tT[:, c, :], psT[c])
            for mc in range(4):
                psS = pL2.tile([128, NODES_PER_CORE], F32, tag="psS")
                for kc in range(4):
                    nc.tensor.matmul(
                        psS,
                        W2sb[:, kc, mc * 128 : (mc + 1) * 128],
                        tT[:, kc, :],
                        start=(kc == 0),
                        stop=(kc == 3),
                    )
                sc = work.tile([128, NODES_PER_CORE], F32, tag="sc")
                nc.vector.tensor_mul(sc, psS, rb)
                nc.vector.tensor_scalar_add(out2T[:, mc, :], sc, b2c[:, mc : mc + 1])

        if DEBUG_STAGE == "l2":
            dbo = work.tile([128, 600], F32, tag="dbo", name="dbo")
            nc.vector.tensor_copy(dbo[:, 0:200], out2T[:, 0, :])
            nc.sync.dma_start(out=d_dbg[:, 0:600], in_=dbo)
            _dbg_out(work, out2T[0:B, 0, 0:2])
            return

        # ------------------------------------------------ FC1 partial + AllReduce
        with tc.tile_pool(name="pFC", bufs=1, space="PSUM") as pFC:
            psZ = pFC.tile([B, ROI], F32, tag="psZ")
            order = [c for fcch in range(4) for c in range(fcch, 100, 4)]
            for i, c in enumerate(order):
                l = c // 4
                fcch = c % 4
                stat = _ap_cols(out2T[:, fcch, :], l, PER_GRAPH, B)
                nc.tensor.matmul(
                    psZ, stat, WfcT[:, c, :], start=(i == 0), stop=(i == 99)
                )
            zsb = work.tile([B, ROI], F32, tag="zsb")
            nc.vector.tensor_copy(zsb, psZ)
            nc.sync.dma_start(out=d_ar_in[:], in_=zsb)

            if DEBUG_STAGE == "fc":
                nc.sync.dma_start(out=d_dbg[0:B, 0:200], in_=zsb)
                _dbg_out(work, zsb[0:B, 0:2])
                return

            nc.gpsimd.collective_compute(
                "AllReduce",
                ALU.add,
                replica_groups=groups,
                ins=[d_ar_in[:]],
                outs=[d_ar_out[:]],
            )

            # ------------------------------------------------ BN + ELU + FC2
            zel = work.tile([100, 2, B], BF16, tag="zel")
            for m in range(2):
                zt = work.tile([100, B], F32, tag="zt")
                nc.sync.dma_start(
                    out=zt,
                    in_=bass.AP(
                        tensor=d_ar_out, offset=m * 100, ap=[[1, 100], [ROI, B]]
                    ),
                )
                nc.vector.tensor_scalar(
                    zt, zt, bnsc[:, m : m + 1], bnsh[:, m : m + 1], ALU.mult, ALU.add
                )
                en = work.tile([100, B], F32, tag="en")
                nc.vector.tensor_scalar_min(en, zt, 0.0)
                nc.scalar.activation(en, en, ACT.Exp)
                nc.vector.tensor_scalar(zt, zt, 0.0, -1.0, ALU.max, ALU.add)
                nc.vector.tensor_add(zt, zt, en)
                nc.vector.tensor_copy(zel[:, m, :], zt)
            psL = pFC.tile([B, 2], F32, tag="psL")
            for m in range(2):
                nc.tensor.matmul(
                    psL, zel[:, m, :], fc2wT[:, m, :], start=(m == 0), stop=(m == 1)
                )
            lsb = work.tile([B, 2], F32, tag="lsb")
            nc.vector.tensor_add(lsb, psL, fc2b)
            nc.sync.dma_start(out=d_out[:], in_=lsb)


# ---------------------------------------------------------------- host side
def _prepare_inputs(x, edge_index, W1, a1_src, a1_dst, b1, W2, a2_src, a2_dst,
                    b2, fc1_w, fc1_b, bn_g, bn_b, bn_m, bn_v, fc2_w, fc2_b):
    x = np.asarray(x, np.float32)
    ei = np.asarray(edge_index)
    src, dst = ei[0].astype(np.int64), ei[1].astype(np.int64)
    C = np.bincount(dst * N + src, minlength=N * N).reshape(N, N).astype(np.float32)
    assert C.max() < 256, "edge multiplicity too large for bf16"

    # pi-order: core j owns, for each graph g, within-graph nodes [25j, 25j+25)
    D = [
        np.array(
            [g * ROI + PER_GRAPH * j + k for g in range(B) for k in range(PER_GRAPH)],
            np.int64,
        )
        for j in range(NCORES)
    ]
    perm = np.concatenate(D)

    W1 = np.asarray(W1, np.float32)
    a1_src = np.asarray(a1_src, np.float32)
    a1_dst = np.asarray(a1_dst, np.float32)
    W2 = np.asarray(W2, np.float32)
    a2_src = np.asarray(a2_src, np.float32)
    a2_dst = np.asarray(a2_dst, np.float32)
    fc1_w = np.asarray(fc1_w, np.float32)

    # Wa[:, h] = W1[:, 64h:64h+64] @ a1_src[h] ; cols 8..16 same with a1_dst
    W1r = W1.reshape(ROI, HIN, HID)
    Wa = np.concatenate(
        [
            np.einsum("rhf,hf->rh", W1r, a1_src),
            np.einsum("rhf,hf->rh", W1r, a1_dst),
        ],
        axis=1,
    )  # [200, 16]
    Wb = np.stack([W2 @ a2_src[0], W2 @ a2_dst[0]], axis=0)  # [2, 512]

    bnscale = np.asarray(bn_g, np.float32) / np.sqrt(np.asarray(bn_v, np.float32) + 1e-5)
    bnshift = (
        np.asarray(bn_b, np.float32)
        + (np.asarray(fc1_b, np.float32) - np.asarray(bn_m, np.float32)) * bnscale
    )

    def _ksw(arr2d, P):
        """[K*P?, F] -> [P, K, F] partition-contiguous swizzle (rows r = k*P + p),
        zero-padding rows beyond the array."""
        R, F = arr2d.shape
        K = (R + P - 1) // P
        out = np.zeros((P, K, F), arr2d.dtype)
        for k in range(K):
            r0, r1 = k * P, min((k + 1) * P, R)
            out[0 : r1 - r0, k, :] = arr2d[r0:r1]
        return out

    xT = _bf(_ksw(x.T, 100))                       # [100, 2, 1600]
    W1_b = _bf(_ksw(W1, 100))                      # [100, 2, 512]
    Wa_b = _bf(_ksw(Wa, 100))                      # [100, 2, 16]
    W2_b = _bf(_ksw(W2, 128))                      # [128, 4, 512]
    fc2wT = _bf(_ksw(np.asarray(fc2_w, np.float32).T, 100))   # [100, 2, 2]
    b2c = _f32(_ksw(np.asarray(b2, np.float32).reshape(-1, 1), 128)[:, :, 0])  # [128, 4]
    bnsc_sw = _f32(_ksw(bnscale.reshape(-1, 1), 100)[:, :, 0])  # [100, 2]
    bnsh_sw = _f32(_ksw(bnshift.reshape(-1, 1), 100)[:, :, 0])  # [100, 2]

    fc1_wr = fc1_w.reshape(ROI, ROI, D1)  # [o, node-in-graph, feat]

    in_maps = []
    for j in range(NCORES):
        Dj = D[j]
        Ct1 = _bf(_ksw(C[Dj, :].T, 128))           # [128, 13, 200]
        Ct2 = _bf(_ksw(C[np.ix_(Dj, perm)].T, 128))
        xTd = _bf(_ksw(x[Dj, :].T, 100))           # [100, 2, 200]
        WfcT = _bf(
            _ksw(
                fc1_wr[:, PER_GRAPH * j : PER_GRAPH * (j + 1), :]
                .transpose(1, 2, 0)
                .reshape(PER_GRAPH * D1, ROI),
                128,
            )
        )                                          # [128, 100, 200]
        in_maps.append(
            {
                "xT": xT,
                "xTd": xTd,
                "W1": W1_b,
                "Wa": Wa_b,
                "Ct1": Ct1,
                "Ct2": Ct2,
                "b1": _f32(np.asarray(b1)),
                "b2c": b2c,
                "Wb": _f32(Wb),
                "W2": W2_b,
                "WfcT": WfcT,
                "bnsc": bnsc_sw,
                "bnsh": bnsh_sw,
                "fc2wT": fc2wT,
                "fc2b": _f32(np.asarray(fc2_b)),
            }
        )
    return in_maps


_CACHE = {}


def kernel(**inputs):
    if "nc" not in _CACHE:
        nc, in_names = build_program()
        _CACHE["nc"] = nc
        _CACHE["in_names"] = in_names
    nc = _CACHE["nc"]
    in_maps = _prepare_inputs(**inputs)
    res = run_bass_kernel_spmd(nc, in_maps, core_ids=list(range(NCORES)))
    _CACHE["last_results"] = res
    return np.asarray(res.results[0]["logits"], np.float32)


if __name__ == "__main__":
    import reference

    inp = {k: np.asarray(v) for k, v in reference.setup_inputs().items()}
    out = kernel(**inp)
    exp = np.asarray(reference.reference(**inp))
    err = np.abs(out - exp).max() / (np.abs(exp).max() + 1e-30)
    print("out:", out)
    print("exp:", exp)
    print("rel err:", err)
